# revision 1
# baseline (speedup 1.0000x reference)
"""Block-diagonal 2x2 equalizer kernel for Trainium2 (8 NeuronCores).

Per point (b, u, s, f) solves the 2x2 system M x = v by Cramer's rule:
    m_ij = h[b, pi[u], i, 0, 2u+j, s, f]   (only 1/4 of h is needed)
    det  = m00*m11 - m01*m10
    x0   = (m11*v0 - m01*v1) / det
    x1   = (m00*v1 - m10*v0) / det
    out[b, u, a, s, f] = x_a

Sharding: data-parallel over batch, 2 batches per core on 8 cores. The host
gathers (precoding_ind) and packs operand planes into contiguous [128, fd]
blocks so every device DMA is a large fully-contiguous transfer.

Device kernel is raw Bass (no TileContext): the neuronxcc walrus used by the
axon/bass2jax path allows only one sync-wait per instruction, so all waits
are standalone wait_ge instructions and every SBUF buffer is written exactly
once (pure dataflow, per-chunk semaphores, no WAR hazards, no tail barrier).

Pipeline (NCH chunks over the u axis):
  sync engine:  per chunk, loads A={m00|m11}, B={m01|m10}, Y={v0|v1}
  DVE:          all 11 tensor ops per chunk (p0, p1, det, q0, q1, r0, q2,
                q3, r1, x0, x1). GPSIMD is intentionally UNUSED: measured
                on HW, concurrent GPSIMD+DVE contend for SBUF ports and
                drop combined throughput below DVE alone (DVE TT 1.09us
                -> 2.9us while GPSIMD runs).
  ACT (scalar): rdet = Reciprocal(det) via direct InstActivation (HW
                spline measured 2.2e-5 max rel err, 1.04us vs 5.75us for
                DVE reciprocal at FD=896); also issues the stores
"""

from contextlib import ExitStack

import numpy as np

import concourse.bass as bass
import concourse.mybir as mybir
from concourse.bass_utils import run_bass_kernel_spmd

# Problem shapes (hardcoded per contract)
B, U, A, NTX, T, S, F = 16, 4, 2, 1, 8, 14, 2048
SF = S * F               # 28672
NCORES = 8
BPC = B // NCORES        # 2 batches per core
NCH = 2                  # pipeline chunks (groups of u)
UPC = U // NCH           # u's per chunk
QW = 448                 # inner width: SF = 64 * 448
ROWS = SF // QW          # 64 rows -> partition p = b*64 + row
FD = UPC * QW            # free elems per component per chunk

# Set by test harness to capture an NTFF profile on the run.
TRACE = False
LAST_RESULTS = None


def _pack(d):
    """[BPC, U, SF] -> [NCH, 128, FD] with p = b*ROWS + sf//QW, f = ul*QW + sf%QW."""
    d = d.reshape(BPC, U, ROWS, QW)
    out = np.empty((NCH, BPC * ROWS, FD), np.float32)
    for k in range(NCH):
        blk = d[:, k * UPC:(k + 1) * UPC]               # [BPC, UPC, ROWS, QW]
        out[k] = blk.transpose(0, 2, 1, 3).reshape(BPC * ROWS, FD)
    return out


def _unpack(t):
    """Inverse of _pack: [NCH, 128, FD] -> [BPC, U, SF]."""
    out = np.empty((BPC, U, ROWS, QW), np.float32)
    for k in range(NCH):
        blk = t[k].reshape(BPC, ROWS, UPC, QW).transpose(0, 2, 1, 3)
        out[:, k * UPC:(k + 1) * UPC] = blk
    return out.reshape(BPC, U, SF)


def _build_nc():
    f32 = mybir.dt.float32
    nc = bass.Bass("TRN2")
    # hA: [m00 | m11], hB: [m01 | m10], yB: [v0 | v1], xout: [x0 | x1]
    hA = nc.dram_tensor("hA", [NCH, 128, 2 * FD], f32, kind="ExternalInput")
    hB = nc.dram_tensor("hB", [NCH, 128, 2 * FD], f32, kind="ExternalInput")
    yB = nc.dram_tensor("yB", [NCH, 128, 2 * FD], f32, kind="ExternalInput")
    xout = nc.dram_tensor("xout", [NCH, 128, 2 * FD], f32, kind="ExternalOutput")

    with ExitStack() as ctx:
        tA = [ctx.enter_context(nc.sbuf_tensor(f"tA{k}", [128, 2 * FD], f32)) for k in range(NCH)]
        tB = [ctx.enter_context(nc.sbuf_tensor(f"tB{k}", [128, 2 * FD], f32)) for k in range(NCH)]
        tY = [ctx.enter_context(nc.sbuf_tensor(f"tY{k}", [128, 2 * FD], f32)) for k in range(NCH)]
        tX = [ctx.enter_context(nc.sbuf_tensor(f"tX{k}", [128, 2 * FD], f32)) for k in range(NCH)]
        tp = [
            {
                n: ctx.enter_context(nc.sbuf_tensor(f"{n}_{k}", [128, FD], f32))
                for n in ("p0", "p1", "q2", "q3", "det", "rdet", "q0", "q1", "r0", "r1")
            }
            for k in range(NCH)
        ]
        semA = [ctx.enter_context(nc.semaphore(f"semA{k}")) for k in range(NCH)]
        semB = [ctx.enter_context(nc.semaphore(f"semB{k}")) for k in range(NCH)]
        semY = [ctx.enter_context(nc.semaphore(f"semY{k}")) for k in range(NCH)]
        semO = [ctx.enter_context(nc.semaphore(f"semO{k}")) for k in range(NCH)]
        dve_sem = ctx.enter_context(nc.semaphore("dve_sem"))
        act_sem = ctx.enter_context(nc.semaphore("act_sem"))

        with nc.Block() as block:

            @block.sync
            def _(sync):
                for k in range(NCH):
                    sync.dma_start(out=tA[k][:], in_=hA[k]).then_inc(semA[k], 16)
                    sync.dma_start(out=tY[k][:], in_=yB[k]).then_inc(semY[k], 16)
                    sync.dma_start(out=tB[k][:], in_=hB[k]).then_inc(semB[k], 16)

            # dve_sem counts: chunk k ops are 11k+1 .. 11k+11
            # order chosen so every consumer is >=2 ops after its producers
            # (a wait_ge on a just-finished DVE op stalls ~1-2us for the
            # producer's pipe DRAIN + sem propagation; with distance the
            # waits are already satisfied): p0 q0 q2 p1 q1 q3 det r0 r1 x0 x1
            @block.vector
            def _(vector):
                for k in range(NCH):
                    a, b, y, x, t = tA[k], tB[k], tY[k], tX[k], tp[k]
                    m00, m11 = a[:, :FD], a[:, FD:]
                    m01, m10 = b[:, :FD], b[:, FD:]
                    v0, v1 = y[:, :FD], y[:, FD:]
                    c = 11 * k
                    vector.wait_ge(semA[k], 16)
                    vector.tensor_mul(t["p0"][:], m00, m11).then_inc(dve_sem, 1)   # c+1
                    vector.wait_ge(semY[k], 16)
                    vector.tensor_mul(t["q0"][:], m11, v0).then_inc(dve_sem, 1)    # c+2
                    vector.tensor_mul(t["q2"][:], m00, v1).then_inc(dve_sem, 1)    # c+3
                    vector.wait_ge(semB[k], 16)
                    vector.tensor_mul(t["p1"][:], m01, m10).then_inc(dve_sem, 1)   # c+4
                    vector.tensor_mul(t["q1"][:], m01, v1).then_inc(dve_sem, 1)    # c+5
                    vector.tensor_mul(t["q3"][:], m10, v0).then_inc(dve_sem, 1)    # c+6
                    vector.wait_ge(dve_sem, c + 4)
                    vector.tensor_sub(t["det"][:], t["p0"][:], t["p1"][:]).then_inc(
                        dve_sem, 1
                    )  # c+7  (ACT recip consumes)
                    vector.wait_ge(dve_sem, c + 5)
                    vector.tensor_sub(t["r0"][:], t["q0"][:], t["q1"][:]).then_inc(
                        dve_sem, 1
                    )  # c+8
                    vector.wait_ge(dve_sem, c + 6)
                    vector.tensor_sub(t["r1"][:], t["q2"][:], t["q3"][:]).then_inc(
                        dve_sem, 1
                    )  # c+9
                    vector.wait_ge(dve_sem, c + 8)
                    vector.wait_ge(act_sem, k + 1)
                    vector.tensor_mul(x[:, :FD], t["r0"][:], t["rdet"][:]).then_inc(
                        dve_sem, 1
                    )  # c+10 (store x0 consumes)
                    vector.wait_ge(dve_sem, c + 9)
                    vector.tensor_mul(x[:, FD:], t["r1"][:], t["rdet"][:]).then_inc(
                        dve_sem, 1
                    )  # c+11 (store x1 consumes)

            @block.scalar
            def _(scalar):
                for k in range(NCH):
                    c = 11 * k
                    scalar.wait_ge(dve_sem, c + 7)
                    scalar.add_instruction(
                        mybir.InstActivation(
                            name=nc.get_next_instruction_name(),
                            func=mybir.ActivationFunctionType.Reciprocal,
                            ins=[
                                scalar.lower_ap(tp[k]["det"][:]),
                                mybir.ImmediateValue(dtype=f32, value=0.0),
                                mybir.ImmediateValue(dtype=f32, value=1.0),
                                mybir.ImmediateValue(dtype=f32, value=0.0),
                            ],
                            outs=[scalar.lower_ap(tp[k]["rdet"][:])],
                        )
                    ).then_inc(act_sem, 1)
                    scalar.wait_ge(dve_sem, c + 10)
                    scalar.dma_start(out=xout[k, :, :FD], in_=tX[k][:, :FD]).then_inc(
                        semO[k], 16
                    )
                    scalar.wait_ge(dve_sem, c + 11)
                    scalar.dma_start(out=xout[k, :, FD:], in_=tX[k][:, FD:]).then_inc(
                        semO[k], 16
                    )
                for k in range(NCH):
                    scalar.wait_ge(semO[k], 32)

    return nc


def make_in_maps(y, h, precoding_ind):
    """Host-side gather + pack. Returns per-core input maps."""
    y = np.asarray(y)
    h = np.asarray(h)
    pi = np.asarray(precoding_ind).astype(np.int64)

    hg = h[:, pi[0]]                                     # [B, U, A, NTX, T, S, F]
    # hsel[b, u, i, j] = hg[b, u, i, 0, 2u+j]  -> components c = i*2+j
    hsel = np.stack(
        [hg[:, u, :, 0, 2 * u:2 * u + 2] for u in range(U)], axis=1
    )                                                    # [B, U, A(i), 2(j), S, F]
    hsel = np.ascontiguousarray(hsel).reshape(B, U, 4, SF).astype(np.float32)
    yr = np.ascontiguousarray(y).reshape(B, U, A, SF).astype(np.float32)

    in_maps = []
    for c in range(NCORES):
        b0 = c * BPC
        hs = hsel[b0:b0 + BPC]                           # [BPC, U, 4, SF]
        ys = yr[b0:b0 + BPC]                             # [BPC, U, A, SF]
        hA = np.concatenate([_pack(hs[:, :, 0]), _pack(hs[:, :, 3])], axis=2)
        hB = np.concatenate([_pack(hs[:, :, 1]), _pack(hs[:, :, 2])], axis=2)
        yB = np.concatenate([_pack(ys[:, :, 0]), _pack(ys[:, :, 1])], axis=2)
        in_maps.append({
            "hA": np.ascontiguousarray(hA),
            "hB": np.ascontiguousarray(hB),
            "yB": np.ascontiguousarray(yB),
        })
    return in_maps


def assemble_output(results):
    """Per-core xout [NCH, 128, 2FD] -> full [B, U, A, S, F]."""
    out = np.empty((B, U, A, S, F), np.float32)
    for c in range(NCORES):
        xo = np.asarray(results[c]["xout"])
        x0 = _unpack(xo[:, :, :FD]).reshape(BPC, U, S, F)
        x1 = _unpack(xo[:, :, FD:]).reshape(BPC, U, S, F)
        out[c * BPC:(c + 1) * BPC, :, 0] = x0
        out[c * BPC:(c + 1) * BPC, :, 1] = x1
    return out


def kernel(y, h, precoding_ind):
    global LAST_RESULTS
    in_maps = make_in_maps(y, h, precoding_ind)
    nc = _build_nc()
    res = run_bass_kernel_spmd(nc, in_maps, list(range(NCORES)), trace=TRACE)
    LAST_RESULTS = res
    return assemble_output(res.results)



# revision 4
# speedup vs baseline: 1.2244x; 1.2244x over previous
"""Block-diagonal 2x2 equalizer kernel for Trainium2 (8 NeuronCores), v2.

Per point (b, u, s, f) solves the 2x2 system M x = v by Cramer's rule:
    m_ij = h[b, pi[u], i, 0, 2u+j, s, f]   (only 1/4 of h is needed)
    det  = m00*m11 - m01*m10
    x0   = (m11*v0 - m01*v1) / det        r0 = q0 - q1
    x1   = (m00*v1 - m10*v0) / det        r1 = q2 - q3

Numerics (validated on host, rel err 3.3e-4 vs gate 2e-2): dets can be as
small as 1.5e-4 with |p| ~ 10, so the det path (m, p0, p1, det) must stay
f32 end to end. Everything else tolerates fp16: y is shipped fp16, the
numerator path (q, r) runs fp16 on DVE at 2x_1P mode, rdet and x are fp16.
This cuts HBM traffic 7.34 -> 5.5 MB/core and halves most DVE op time.

Sharding: data parallel over batch, 2 batches per core on 8 cores. Host
gathers (precoding_ind indexing only) and packs column-chunked planes:
  partition p = b_local*64 + sf//448, column c = u*448 + sf%448  (1792 cols)
  ha = [m11|m00] f32, hb = [m01|m10] f32, yb = [v0|v1] fp16 (per chunk)

Device graph per chunk (W = chunk width in base columns):
  DVE:  p0 = m11*m00 (f32 1x), p1 = m01*m10, det = p0-p1,
        Q = cA.*yb -> [q0|q2] (fp16 2x), R = cB.*rev(yb) -> [q1|q3],
        RR = Q-R -> [r0|r1], X = RR.*[rdet|rdet] -> [x0|x1]
  ACT:  cA = fp16(ha), cB = fp16(hb), rdet = Recip(det) written twice
        ([rdet|rdet] contiguous so X stays in 2x mode); dummy recip at t=0
        preloads the activation table off the critical path
  sync: ha/hb/yb loads (ha split so p0 can start before hb lands)
  ACT also issues the X stores.

Raw Bass (no TileContext): the neuronxcc walrus used by the axon/bass2jax
path allows only one sync-wait per instruction, so all waits are standalone
wait_ge instructions; every SBUF buffer is written exactly once (pure
dataflow). Same-engine RAW needs no semaphore (program order + DVE DRAIN),
so dve_sem/act_sem only guard cross-engine edges, placed >=2 producer ops
back where possible to hide sem propagation latency.
"""

from contextlib import ExitStack

import numpy as np

import concourse.bass as bass
import concourse.mybir as mybir
from concourse.bass_utils import run_bass_kernel_spmd

# Problem shapes (hardcoded per contract)
B, U, A, NTX, T, S, F = 16, 4, 2, 1, 8, 14, 2048
SF = S * F               # 28672
NCORES = 8
BPC = B // NCORES        # 2 batches per core
QW = 448                 # SF = 64 * 448; partition p = b*64 + sf//448
ROWS = SF // QW          # 64
COLS = U * QW            # 1792 base columns per core
WIDTHS = [448, 896, 448]  # column chunks (sum = COLS)
NCH = len(WIDTHS)
OFFS = [sum(WIDTHS[:k]) for k in range(NCH)]

TRACE = False
LAST_RESULTS = None


def _to_cols(d):
    """[BPC, U, SF] -> [128, COLS] with p = b*64 + sf//448, c = u*448 + sf%448."""
    d = d.reshape(BPC, U, ROWS, QW).transpose(0, 2, 1, 3)
    return np.ascontiguousarray(d).reshape(BPC * ROWS, COLS)


def _from_cols(m):
    """Inverse of _to_cols: [128, COLS] -> [BPC, U, SF]."""
    d = m.reshape(BPC, ROWS, U, QW).transpose(0, 2, 1, 3)
    return np.ascontiguousarray(d).reshape(BPC, U, SF)


def _build_nc():
    f32 = mybir.dt.float32
    f16 = mybir.dt.float16
    nc = bass.Bass("TRN2")

    dha = [nc.dram_tensor(f"ha{k}", [128, 2, W], f32, kind="ExternalInput")
           for k, W in enumerate(WIDTHS)]
    dhb = [nc.dram_tensor(f"hb{k}", [128, 2, W], f32, kind="ExternalInput")
           for k, W in enumerate(WIDTHS)]
    dyb = [nc.dram_tensor(f"yb{k}", [128, 2, W], f16, kind="ExternalInput")
           for k, W in enumerate(WIDTHS)]
    dx = [nc.dram_tensor(f"xout{k}", [128, 2, W], f16, kind="ExternalOutput")
          for k, W in enumerate(WIDTHS)]

    with ExitStack() as ctx:
        sb = lambda n, shp, dt: ctx.enter_context(nc.sbuf_tensor(n, shp, dt))
        tHa = [sb(f"tHa{k}", [128, 2, W], f32) for k, W in enumerate(WIDTHS)]
        tHb = [sb(f"tHb{k}", [128, 2, W], f32) for k, W in enumerate(WIDTHS)]
        tY = [sb(f"tY{k}", [128, 2, W], f16) for k, W in enumerate(WIDTHS)]
        cA = [sb(f"cA{k}", [128, 2, W], f16) for k, W in enumerate(WIDTHS)]
        cB = [sb(f"cB{k}", [128, 2, W], f16) for k, W in enumerate(WIDTHS)]
        tp0 = [sb(f"p0_{k}", [128, W], f32) for k, W in enumerate(WIDTHS)]
        tp1 = [sb(f"p1_{k}", [128, W], f32) for k, W in enumerate(WIDTHS)]
        tdet = [sb(f"det{k}", [128, W], f32) for k, W in enumerate(WIDTHS)]
        trd = [sb(f"rd{k}", [128, 2, W], f16) for k, W in enumerate(WIDTHS)]
        tQ = [sb(f"Q{k}", [128, 2, W], f16) for k, W in enumerate(WIDTHS)]
        tR = [sb(f"R{k}", [128, 2, W], f16) for k, W in enumerate(WIDTHS)]
        tRR = [sb(f"RR{k}", [128, 2, W], f16) for k, W in enumerate(WIDTHS)]
        tX = [sb(f"X{k}", [128, 2, W], f16) for k, W in enumerate(WIDTHS)]
        scr_in = sb("scr_in", [128, 8], f32)
        scr_out = sb("scr_out", [128, 8], f32)

        semHa = [ctx.enter_context(nc.semaphore(f"semHa{k}")) for k in range(NCH)]
        semHb = [ctx.enter_context(nc.semaphore(f"semHb{k}")) for k in range(NCH)]
        semY = [ctx.enter_context(nc.semaphore(f"semY{k}")) for k in range(NCH)]
        semO = [ctx.enter_context(nc.semaphore(f"semO{k}")) for k in range(NCH)]
        dve_sem = ctx.enter_context(nc.semaphore("dve_sem"))
        act_sem = ctx.enter_context(nc.semaphore("act_sem"))

        # Precomputed 1-based semaphore targets (vector body is emitted before
        # scalar body, so cross-engine indices cannot be recorded on the fly).
        # ACT order/chunk: cvtA, cvtB, recipA, recipB  -> 4 sem-incs per chunk
        act_cvtA = [4 * k + 1 for k in range(NCH)]
        act_cvtB = [4 * k + 2 for k in range(NCH)]
        act_recipB = [4 * k + 4 for k in range(NCH)]
        # DVE order: k=0: p0,p1,det,Q,R,RR; k>=1: p0,p1,det,X(k-1),Q,R,RR; X(last)
        dve_det = [3 if k == 0 else 7 * k + 2 for k in range(NCH)]
        dve_X = [7 * k + 10 for k in range(NCH - 1)] + [7 * NCH]
        dve_i = {"n": 0}

        def recip(scalar, in_ap, out_ap):
            return scalar.add_instruction(
                mybir.InstActivation(
                    name=nc.get_next_instruction_name(),
                    func=mybir.ActivationFunctionType.Reciprocal,
                    ins=[
                        scalar.lower_ap(in_ap),
                        mybir.ImmediateValue(dtype=mybir.dt.float32, value=0.0),
                        mybir.ImmediateValue(dtype=mybir.dt.float32, value=1.0),
                        mybir.ImmediateValue(dtype=mybir.dt.float32, value=0.0),
                    ],
                    outs=[scalar.lower_ap(out_ap)],
                )
            )

        with nc.Block() as block:

            @block.sync
            def _(sync):
                for k in range(NCH):
                    sync.dma_start(out=tHa[k][:], in_=dha[k][:]).then_inc(semHa[k], 16)
                    sync.dma_start(out=tHb[k][:], in_=dhb[k][:]).then_inc(semHb[k], 16)
                    sync.dma_start(out=tY[k][:], in_=dyb[k][:]).then_inc(semY[k], 16)

            @block.vector
            def _(vector):
                def op(f, *a):
                    f(*a).then_inc(dve_sem, 1)
                    dve_i["n"] += 1
                    return dve_i["n"]

                for k in range(NCH):
                    # det path (f32)
                    vector.wait_ge(semHa[k], 16)
                    n = op(vector.tensor_mul, tp0[k][:], tHa[k][:, 0], tHa[k][:, 1])
                    vector.wait_ge(semHb[k], 16)
                    op(vector.tensor_mul, tp1[k][:], tHb[k][:, 0], tHb[k][:, 1])
                    n = op(vector.tensor_sub, tdet[k][:], tp0[k][:], tp1[k][:])
                    assert n == dve_det[k]
                    # previous chunk's X now that its rdet is surely ready
                    if k > 0:
                        vector.wait_ge(act_sem, act_recipB[k - 1])
                        n = op(
                            vector.tensor_mul, tX[k - 1][:], tRR[k - 1][:], trd[k - 1][:]
                        )
                        assert n == dve_X[k - 1]
                    # numerator path (fp16, 2x mode)
                    vector.wait_ge(semY[k], 16)
                    vector.wait_ge(act_sem, act_cvtA[k])
                    op(vector.tensor_mul, tQ[k][:], cA[k][:], tY[k][:])
                    vector.wait_ge(act_sem, act_cvtB[k])
                    op(vector.tensor_mul, tR[k][:], cB[k][:], tY[k][:, ::-1, :])
                    op(vector.tensor_sub, tRR[k][:], tQ[k][:], tR[k][:])
                last = NCH - 1
                vector.wait_ge(act_sem, act_recipB[last])
                n = op(vector.tensor_mul, tX[last][:], tRR[last][:], trd[last][:])
                assert n == dve_X[last]

            @block.scalar
            def _(scalar):
                nact = {"n": 0}

                def aop(inst):
                    inst.then_inc(act_sem, 1)
                    nact["n"] += 1
                    return nact["n"]

                # preload the activation table set while DMA streams in
                # (reads uninitialized scratch; result is never consumed)
                recip(scalar, scr_in[:], scr_out[:])

                for k in range(NCH):
                    scalar.wait_ge(semHa[k], 16)
                    n = aop(scalar.copy(cA[k][:], tHa[k][:]))
                    assert n == act_cvtA[k]
                    scalar.wait_ge(semHb[k], 16)
                    n = aop(scalar.copy(cB[k][:], tHb[k][:]))
                    assert n == act_cvtB[k]
                    scalar.wait_ge(dve_sem, dve_det[k])
                    aop(recip(scalar, tdet[k][:], trd[k][:, 0]))
                    n = aop(recip(scalar, tdet[k][:], trd[k][:, 1]))
                    assert n == act_recipB[k]
                    if k > 0:
                        scalar.wait_ge(dve_sem, dve_X[k - 1])
                        scalar.dma_start(out=dx[k - 1][:], in_=tX[k - 1][:]).then_inc(
                            semO[k - 1], 16
                        )
                last = NCH - 1
                scalar.wait_ge(dve_sem, dve_X[last])
                scalar.dma_start(out=dx[last][:], in_=tX[last][:]).then_inc(
                    semO[last], 16
                )
                for k in range(NCH):
                    scalar.wait_ge(semO[k], 16)

    return nc


def make_in_maps(y, h, precoding_ind):
    """Host-side gather + pack (indexing/layout only; no arithmetic)."""
    y = np.asarray(y)
    h = np.asarray(h)
    pi = np.asarray(precoding_ind).astype(np.int64)

    hg = h[:, pi[0]]                                     # [B, U, A, NTX, T, S, F]
    # hsel[b, u, i, j] = hg[b, u, i, 0, 2u+j]
    hsel = np.stack(
        [hg[:, u, :, 0, 2 * u:2 * u + 2] for u in range(U)], axis=1
    )                                                    # [B, U, 2(i), 2(j), S, F]
    hsel = np.ascontiguousarray(hsel).reshape(B, U, 4, SF).astype(np.float32)
    yr = np.ascontiguousarray(y).reshape(B, U, A, SF)

    in_maps = []
    for c in range(NCORES):
        b0 = c * BPC
        hs = hsel[b0:b0 + BPC]                           # [BPC, U, 4, SF]
        ys = yr[b0:b0 + BPC]                             # [BPC, U, 2, SF]
        m00 = _to_cols(hs[:, :, 0])
        m01 = _to_cols(hs[:, :, 1])
        m10 = _to_cols(hs[:, :, 2])
        m11 = _to_cols(hs[:, :, 3])
        v0 = _to_cols(ys[:, :, 0]).astype(np.float16)
        v1 = _to_cols(ys[:, :, 1]).astype(np.float16)
        mp = {}
        for k, (o, W) in enumerate(zip(OFFS, WIDTHS)):
            mp[f"ha{k}"] = np.ascontiguousarray(
                np.stack([m11[:, o:o + W], m00[:, o:o + W]], axis=1))
            mp[f"hb{k}"] = np.ascontiguousarray(
                np.stack([m01[:, o:o + W], m10[:, o:o + W]], axis=1))
            mp[f"yb{k}"] = np.ascontiguousarray(
                np.stack([v0[:, o:o + W], v1[:, o:o + W]], axis=1))
        in_maps.append(mp)
    return in_maps


def assemble_output(results):
    out = np.empty((B, U, A, S, F), np.float32)
    for c in range(NCORES):
        x0 = np.empty((128, COLS), np.float32)
        x1 = np.empty((128, COLS), np.float32)
        for k, (o, W) in enumerate(zip(OFFS, WIDTHS)):
            xo = np.asarray(results[c][f"xout{k}"]).astype(np.float32)
            x0[:, o:o + W] = xo[:, 0]
            x1[:, o:o + W] = xo[:, 1]
        b0 = c * BPC
        out[b0:b0 + BPC, :, 0] = _from_cols(x0).reshape(BPC, U, S, F)
        out[b0:b0 + BPC, :, 1] = _from_cols(x1).reshape(BPC, U, S, F)
    return out


def kernel(y, h, precoding_ind):
    global LAST_RESULTS
    in_maps = make_in_maps(y, h, precoding_ind)
    nc = _build_nc()
    res = run_bass_kernel_spmd(nc, in_maps, list(range(NCORES)), trace=TRACE)
    LAST_RESULTS = res
    return assemble_output(res.results)
